# revision 30
# baseline (speedup 1.0000x reference)
"""Trainium2 kernel for nn_ConservationOfFeatureSimilarity.

Math (see reference): with xn = row-normalized feature embeddings (M, 256) and
zn = row-normalized frozen embeddings (M, 768), M = B*N = 3136:

  feat_sim  = xn @ xn.T        (M, M)
  frozen_sim= zn @ zn.T        (M, M)
  ranking   = triu+ * (feat-frozen) * [cls_i != cls_j] * [pidx_i == pidx_j] * mps_i*mps_j
  top5      = top_k(ranking.flat, 5);  sel rows/cols
  out       = mean |feat_sim[sel] - frozen_sim[sel]|  over (5, 2, M)
            = (sum over the 10 selected row indices of S[r]) / (10*M)
  where S_i = sum_j |feat_sim[i,j] - frozen_sim[i,j]|.

Device (8 NeuronCores): the dense O(M^2 * K) part — S row sums — runs as fp8
(e4m3) DoubleRow matmuls: embeddings are scaled by 16, quantized to fp8, and
diff tiles accumulate in PSUM over 4 DoubleRow groups (256-deep contraction
each; frozen chunks sign-flipped on the weights side so one gram matrix is
subtracted). |diff| is symmetric, so only blocks at-or-right-of the diagonal
of a (25 row-tile x 13 col-block) grid are computed: rows padded to
3200 = 25*128, cols to 3328 = 13*256. Core c owns row tiles {8t+c: t=0..3};
slot t computes col blocks c >= 4t (28 blocks/core; 13+9+5+1). Each block
contributes DVE row sums, and strictly-upper blocks contribute mirrored
column sums via a masked ones-matmul (the per-core 0/1 mask is data, not
code). The host drops the few below-diagonal row-sum partials.

Host: normalization/transposes, fp8 quantization, prototype argmax, the top-5
search (ranking is nonzero only for same-argmax-prototype pairs: ~25K of the
9.8M pairs, so it is evaluated sparsely in numpy), and the final combine.
"""

import sys

if "/opt/trn_rl_repo" not in sys.path:
    sys.path.insert(0, "/opt/trn_rl_repo")

import numpy as np
import ml_dtypes

BF16 = ml_dtypes.bfloat16
F8 = ml_dtypes.float8_e4m3

B, N, D, NF, P = 16, 196, 768, 256, 200
M = B * N                      # 3136
NCORES = 8
RT = 128                       # row tile height
NRT = 25                       # row tiles (rows padded to 3200)
MR = RT * NRT                  # 3200
CBW = 256                      # col block width
NCB = 13                       # col blocks (cols padded to 3328)
MC = CBW * NCB                 # 3328
NK = 8                         # 128-deep K chunks: 2 feat + 6 frozen
NG = 4                         # DoubleRow groups (256-deep each)
NSLOT = 4                      # row tiles per core (slot t -> tile 8t+c)
CSTART = (0, 4, 8, 12)         # first col block computed by slot t
SCALE = 16.0                   # fp8 quantization scale
K_ = 5
GAMMA = 1.0
EPS = 1e-8

# blocks processed in PAIRS: two 256-col blocks share one PSUM bank, one
# 512-wide abs, and one dual row-sum reduce. Mostly same-slot c-adjacent
# pairs (weight reuse); the four c=12 stragglers pair across slots.
# descending-c order: the first-arriving col bands feed 3 slots' worth of
# compute each, so the PE never starves once the first band lands
PAIRS = [
    ((2, 10), (2, 11)), ((1, 10), (1, 11)), ((0, 10), (0, 11)),
    ((2, 8), (2, 9)), ((1, 8), (1, 9)), ((0, 8), (0, 9)),
    ((0, 12), (1, 12)), ((2, 12), (3, 12)),
    ((1, 6), (1, 7)), ((0, 6), (0, 7)),
    ((1, 4), (1, 5)), ((0, 4), (0, 5)),
    ((0, 2), (0, 3)), ((0, 0), (0, 1)),
]
BLOCKS = [blk for pr in PAIRS for blk in pr]
NB = len(BLOCKS)               # 28
NMIR = sum(1 for t, c in BLOCKS if c > CSTART[t])  # 24 mirrored blocks

_COMPILED = None
_last_bass_results = None


def _build():
    from concourse import bacc, mybir
    import concourse.tile as tile

    f32 = mybir.dt.float32
    bf16 = mybir.dt.bfloat16
    fp8 = mybir.dt.float8e4
    DR = mybir.MatmulPerfMode.DoubleRow
    nc = bacc.Bacc("TRN2", target_bir_lowering=False, debug=False,
                   num_devices=NCORES)

    # wts[t]: slot t's row tile, [K-part, chunk, row] with frozen chunks
    # negated; cols[c]: col band c, [K-part, chunk, col]; both fp8.
    wts = nc.declare_dram_parameter("wts", [NSLOT, 128, NK, RT], fp8,
                                    isOutput=False)
    cols = nc.declare_dram_parameter("cols", [NCB, 128, NK, CBW], fp8,
                                     isOutput=False)
    cmask = nc.declare_dram_parameter("cmask", [128, NB * 16], bf16,
                                      isOutput=False)
    racc_out = nc.declare_dram_parameter("racc", [128, NB], f32,
                                         isOutput=True)
    cs_out = nc.declare_dram_parameter("cs", [16, CBW], f32, isOutput=True)

    with tile.TileContext(nc) as tc:
        with (
            tc.tile_pool(name="inp", bufs=1) as inp,
            tc.tile_pool(name="pd", bufs=7, space="PSUM") as pd,
            tc.tile_pool(name="pcs", bufs=1, space="PSUM") as pcs,
            tc.tile_pool(name="adp", bufs=6) as adp,
            tc.tile_pool(name="outp", bufs=1) as outp,
        ):
            # Input DMAs on three queues (each sustains ~220 GB/s): col
            # bands alternate sync/gpsimd in compute order; weights + mask
            # ride the scalar queue, which is otherwise idle early.
            wt_t = []
            for t in range(NSLOT):
                t_ = inp.tile([128, NK, RT], fp8, name=f"wt{t}", tag=f"wt{t}")
                wt_t.append(t_)
            col_t = []
            for c in range(NCB):
                t_ = inp.tile([128, NK, CBW], fp8, name=f"col{c}",
                              tag=f"col{c}")
                col_t.append(t_)
            cm_t = inp.tile([128, NB * 16], bf16, name="cm_t", tag="cm_t")

            warm_s = inp.tile([128, CBW], bf16, name="warm_s", tag="warm_s")
            nc.vector.memset(warm_s[:], 0.0)
            nc.scalar.dma_start(wt_t[2][:], wts[2])
            nc.scalar.dma_start(cm_t[:], cmask[:])
            nc.scalar.dma_start(wt_t[1][:], wts[1])
            nc.scalar.dma_start(wt_t[0][:], wts[0])
            nc.scalar.dma_start(wt_t[3][:], wts[3])
            # col10/col11 lead in halves (parallel DMA channels -> earlier
            # first pair); evens on sync, odds on gpsimd, in compute order
            nc.sync.dma_start(col_t[10][:, :4, :], cols[10][:, :4, :])
            nc.sync.dma_start(col_t[10][:, 4:, :], cols[10][:, 4:, :])
            nc.gpsimd.dma_start(col_t[11][:, :4, :], cols[11][:, :4, :])
            nc.gpsimd.dma_start(col_t[11][:, 4:, :], cols[11][:, 4:, :])
            for c in (8, 12, 6, 4, 2, 0):
                nc.sync.dma_start(col_t[c][:], cols[c])
            for c in (9, 7, 5, 3, 1):
                nc.gpsimd.dma_start(col_t[c][:], cols[c])

            racc_t = outp.tile([128, NB], f32, name="racc_t", tag="racc_t")
            cs_psum = pcs.tile([16, CBW], f32, name="cs_psum", tag="cs_psum")

            # PE warm-up during the DMA wait: ramp the clock
            for w in range(6):
                warm_p = pd.tile([128, 2, CBW], f32, name=f"warm{w}", tag="d")
                nc.tensor.matmul(warm_p[:, 0, :], warm_s[:, :128], warm_s[:],
                                 start=True, stop=True)

            nmir = 0
            pend = []          # deferred mirror matmuls: (b, ad2, j)

            def flush_cs(last=False):
                nonlocal nmir
                for b_, ad2_, j_ in pend:
                    nc.tensor.matmul(
                        cs_psum[:],
                        cm_t[:, 16 * b_: 16 * (b_ + 1)],
                        ad2_[:, j_, :],
                        start=(nmir == 0),
                        stop=(last and nmir == NMIR - 1),
                    )
                    nmir += 1
                pend.clear()

            for pi, (blkA, blkB) in enumerate(PAIRS):
                d2 = pd.tile([128, 2, CBW], f32, name=f"d2_{pi}", tag="d")
                # one PSUM accumulation group over both halves of the bank:
                # start zeroes the whole bank (pending-zero), halves then
                # accumulate independently
                for g in range(NG):
                    for j, (t, c) in enumerate((blkA, blkB)):
                        nc.tensor.matmul(
                            d2[:, j, :],
                            wt_t[t][:, 2 * g: 2 * g + 2, :],
                            col_t[c][:, 2 * g: 2 * g + 2, :],
                            start=(g == 0 and j == 0),
                            stop=(g == NG - 1 and j == 1),
                            perf_mode=DR,
                            skip_group_check=True,
                        )
                ad2 = adp.tile([128, 2, CBW], bf16, name=f"ad2_{pi}",
                               tag="ad")
                nc.scalar.activation(ad2[:], d2[:],
                                     mybir.ActivationFunctionType.Abs)
                nc.vector.tensor_reduce(
                    out=racc_t[:, 2 * pi: 2 * pi + 2],
                    in_=ad2[:],
                    axis=mybir.AxisListType.X,
                    op=mybir.AluOpType.add,
                )
                for j, (t, c) in enumerate((blkA, blkB)):
                    if c > CSTART[t]:
                        pend.append((2 * pi + j, ad2, j))
                # batch mirror matmuls of 3 pairs: one weight-switch
                # round-trip on the PE per batch instead of per pair
                if pi % 3 == 2:
                    flush_cs()
            flush_cs(last=True)

            cs_sb = outp.tile([16, CBW], f32, name="cs_sb", tag="cs_sb")
            nc.scalar.copy(cs_sb[:], cs_psum[:])
            nc.sync.dma_start(cs_out[:], cs_sb[:])
            nc.sync.dma_start(racc_out[:], racc_t[:])

    nc.compile()
    return nc


def _get_compiled():
    global _COMPILED
    if _COMPILED is None:
        _COMPILED = _build()
    return _COMPILED


def _normalize(x):
    n = np.sqrt((x.astype(np.float64) ** 2).sum(-1, keepdims=True))
    return (x / np.maximum(n, EPS)).astype(np.float32)


def _device_rowsums(xnf, xnz):
    """xnf (M, 256), xnz (M, 768) f32 -> S (M,) row sums of |feat-frozen|."""
    global _last_bass_results
    from concourse.bass_utils import run_bass_kernel_spmd

    nc = _get_compiled()

    qf = (xnf * SCALE).astype(F8)                 # (M, 256)
    qz = (xnz * SCALE).astype(F8)                 # (M, 768)
    # K-major chunks, cols zero-padded to MC
    chunks = np.zeros((NK, 128, MC), F8)
    chunks[:2, :, :M] = np.ascontiguousarray(qf.T).reshape(2, 128, M)
    chunks[2:, :, :M] = np.ascontiguousarray(qz.T).reshape(6, 128, M)
    # cols[c, p, k, x] = chunks[k, p, CBW*c + x]
    cols_np = np.ascontiguousarray(
        chunks.reshape(NK, 128, NCB, CBW).transpose(2, 1, 0, 3))
    # weights: frozen chunks sign-flipped; rows use the MR (=3200) padding
    wneg = chunks[:, :, :MR].copy()
    wneg[2:] = (wneg[2:].view(np.uint8) ^ 0x80).view(F8)
    wall = np.ascontiguousarray(
        wneg.reshape(NK, 128, NRT, RT).transpose(2, 1, 0, 3))  # [25,128,8,128]

    in_maps = []
    for cid in range(NCORES):
        wt = np.zeros((NSLOT, 128, NK, RT), F8)
        cm = np.zeros((128, NB, 16), np.float32)
        for t in range(NSLOT):
            r = NCORES * t + cid
            if r < NRT:
                wt[t] = wall[r]
                jd = r // 2
                for b, (bt, c) in enumerate(BLOCKS):
                    if bt == t and c > jd:
                        cm[:, b, c] = 1.0
        in_maps.append({
            "wts": wt,
            "cols": cols_np,
            "cmask": np.ascontiguousarray(
                cm.reshape(128, NB * 16)).astype(BF16),
        })

    res = run_bass_kernel_spmd(nc, in_maps, list(range(NCORES)))
    _last_bass_results = res

    S = np.zeros(MC, np.float64)
    for cid in range(NCORES):
        racc = res.results[cid]["racc"].astype(np.float64)   # (128, 28)
        cs = res.results[cid]["cs"].astype(np.float64)       # (16, 256)
        for b, (t, c) in enumerate(BLOCKS):
            r = NCORES * t + cid
            if r < NRT and c >= r // 2:
                S[RT * r: RT * (r + 1)] += racc[:, b]
        S[:MC] += cs[:NCB].reshape(-1)
    return (S[:M] / (SCALE * SCALE)).astype(np.float32)


def kernel(frozen_embeddings, feature_embeddings, proto_sim, labels):
    fz = np.asarray(frozen_embeddings, dtype=np.float32).reshape(M, D)
    fn = np.asarray(feature_embeddings, dtype=np.float32).reshape(M, NF)
    ps_ = np.asarray(proto_sim, dtype=np.float32)
    lab = np.asarray(labels)

    xnf = _normalize(fn)
    xnz = _normalize(fz)

    # dense part on the 8 NeuronCores
    S = _device_rowsums(xnf, xnz)

    # prototype max/argmax and labels (host, tiny)
    psr = ps_.transpose(0, 2, 1).reshape(M, P)
    mps = psr.max(1)
    pidx = psr.argmax(1)
    ext = np.repeat(lab, N)

    # sparse ranking candidates: only same-argmax-prototype pairs can be nonzero
    cand_vals, cand_flat = [], []
    for p in np.unique(pidx):
        g = np.nonzero(pidx == p)[0]
        s = len(g)
        if s < 2:
            continue
        F = xnf[g] @ xnf[g].T
        Z = xnz[g] @ xnz[g].T
        V = (F - Z) * np.outer(mps[g], mps[g])
        iu, ju = np.triu_indices(s, 1)
        ok = ext[g][iu] != ext[g][ju]
        if ok.any():
            cand_vals.append(V[iu[ok], ju[ok]].astype(np.float64))
            cand_flat.append(g[iu[ok]].astype(np.int64) * M + g[ju[ok]])
    if cand_vals:
        vals = np.concatenate(cand_vals)
        flats = np.concatenate(cand_flat)
    else:
        vals = np.zeros(0)
        flats = np.zeros(0, np.int64)

    # top-5 with lax.top_k tie semantics (desc value, then asc flat index);
    # entries not in the candidate set are exact zeros in the ranking matrix.
    order = np.lexsort((flats, -vals))
    pos = [f for f in order if vals[f] > 0][:K_]
    sel_flats = [int(flats[i]) for i in pos]
    if len(sel_flats) < K_:
        nonzero = set(int(f) for v, f in zip(vals, flats) if v != 0.0)
        f = 0
        while len(sel_flats) < K_:
            if f not in nonzero:
                sel_flats.append(f)
            f += 1
    sel_flats = np.asarray(sel_flats, np.int64)
    rows = sel_flats // M
    cols_sel = sel_flats % M

    out = GAMMA * (S[rows].sum(dtype=np.float64)
                   + S[cols_sel].sum(dtype=np.float64)) / (2 * K_ * M)
    return np.asarray(np.float32(out))


# revision 33
# speedup vs baseline: 1.0719x; 1.0719x over previous
"""Trainium2 kernel for nn_ConservationOfFeatureSimilarity.

Math (see reference): with xn = row-normalized feature embeddings (M, 256) and
zn = row-normalized frozen embeddings (M, 768), M = B*N = 3136:

  feat_sim  = xn @ xn.T        (M, M)
  frozen_sim= zn @ zn.T        (M, M)
  ranking   = triu+ * (feat-frozen) * [cls_i != cls_j] * [pidx_i == pidx_j] * mps_i*mps_j
  top5      = top_k(ranking.flat, 5);  sel rows/cols
  out       = mean |feat_sim[sel] - frozen_sim[sel]|  over (5, 2, M)
            = (sum over the 10 selected row indices of S[r]) / (10*M)
  where S_i = sum_j |feat_sim[i,j] - frozen_sim[i,j]|.

Device (8 NeuronCores): the dense O(M^2 * K) part — S row sums — runs as fp8
(e4m3) DoubleRow matmuls: embeddings are scaled by 16, quantized to fp8, and
diff tiles accumulate in PSUM over 4 DoubleRow groups (256-deep contraction
each; frozen chunks sign-flipped on the weights side so one gram matrix is
subtracted). |diff| is symmetric, so only blocks at-or-right-of the diagonal
of a (25 row-tile x 13 col-block) grid are computed: rows padded to
3200 = 25*128, cols to 3328 = 13*256. Core c owns row tiles {8t+c: t=0..3};
slot t computes col blocks c >= 4t (28 blocks/core; 13+9+5+1). Each block
contributes DVE row sums, and strictly-upper blocks contribute mirrored
column sums via a masked ones-matmul (the per-core 0/1 mask is data, not
code). The host drops the few below-diagonal row-sum partials.

Host: normalization/transposes, fp8 quantization, prototype argmax, the top-5
search (ranking is nonzero only for same-argmax-prototype pairs: ~25K of the
9.8M pairs, so it is evaluated sparsely in numpy), and the final combine.
"""

import sys

if "/opt/trn_rl_repo" not in sys.path:
    sys.path.insert(0, "/opt/trn_rl_repo")

import numpy as np
import ml_dtypes

BF16 = ml_dtypes.bfloat16
F8 = ml_dtypes.float8_e4m3

B, N, D, NF, P = 16, 196, 768, 256, 200
M = B * N                      # 3136
NCORES = 8
RT = 128                       # row tile height
NRT = 25                       # row tiles (rows padded to 3200)
MR = RT * NRT                  # 3200
CBW = 256                      # col block width
NCB = 13                       # col blocks (cols padded to 3328)
MC = CBW * NCB                 # 3328
NK = 8                         # 128-deep K chunks: 2 feat + 6 frozen
NG = 4                         # DoubleRow groups (256-deep each)
NSLOT = 4                      # row tiles per core (slot t -> tile 8t+c)
CSTART = (0, 4, 8, 12)         # first col block computed by slot t
SCALE = 16.0                   # fp8 quantization scale
K_ = 5
GAMMA = 1.0
EPS = 1e-8

# blocks processed in PAIRS: two 256-col blocks share one PSUM bank, one
# 512-wide abs, and one dual row-sum reduce. Mostly same-slot c-adjacent
# pairs (weight reuse); the four c=12 stragglers pair across slots.
PAIRS = [
    ((0, 0), (0, 1)), ((0, 2), (0, 3)),
    ((0, 4), (0, 5)), ((1, 4), (1, 5)),
    ((0, 6), (0, 7)), ((1, 6), (1, 7)),
    ((0, 8), (0, 9)), ((1, 8), (1, 9)), ((2, 8), (2, 9)),
    ((0, 10), (0, 11)), ((1, 10), (1, 11)), ((2, 10), (2, 11)),
    ((0, 12), (1, 12)), ((2, 12), (3, 12)),
]
BLOCKS = [blk for pr in PAIRS for blk in pr]
NB = len(BLOCKS)               # 28
NMIR = sum(1 for t, c in BLOCKS if c > CSTART[t])  # 24 mirrored blocks

_COMPILED = None
_last_bass_results = None


def _build():
    from concourse import bacc, mybir
    import concourse.tile as tile

    f32 = mybir.dt.float32
    bf16 = mybir.dt.bfloat16
    fp8 = mybir.dt.float8e4
    DR = mybir.MatmulPerfMode.DoubleRow
    nc = bacc.Bacc("TRN2", target_bir_lowering=False, debug=False,
                   num_devices=NCORES)

    # wts[t]: slot t's row tile, [K-part, chunk, row] with frozen chunks
    # negated; cols[c]: col band c, [K-part, chunk, col]; both fp8.
    wts = nc.declare_dram_parameter("wts", [NSLOT, 128, NK, RT], fp8,
                                    isOutput=False)
    cols = nc.declare_dram_parameter("cols", [NCB, 128, NK, CBW], fp8,
                                     isOutput=False)
    cmask = nc.declare_dram_parameter("cmask", [128, NB * 16], bf16,
                                      isOutput=False)
    racc_out = nc.declare_dram_parameter("racc", [128, NB], f32,
                                         isOutput=True)
    cs_out = nc.declare_dram_parameter("cs", [16, CBW], f32, isOutput=True)

    with tile.TileContext(nc) as tc:
        with (
            tc.tile_pool(name="inp", bufs=1) as inp,
            tc.tile_pool(name="pd", bufs=7, space="PSUM") as pd,
            tc.tile_pool(name="pcs", bufs=1, space="PSUM") as pcs,
            tc.tile_pool(name="adp", bufs=4) as adp,
            tc.tile_pool(name="outp", bufs=1) as outp,
        ):
            # Input DMAs on three queues (each sustains ~220 GB/s): col
            # bands alternate sync/gpsimd in compute order; weights + mask
            # ride the scalar queue, which is otherwise idle early.
            wt_t = []
            for t in range(NSLOT):
                t_ = inp.tile([128, NK, RT], fp8, name=f"wt{t}", tag=f"wt{t}")
                wt_t.append(t_)
            col_t = []
            for c in range(NCB):
                t_ = inp.tile([128, NK, CBW], fp8, name=f"col{c}",
                              tag=f"col{c}")
                col_t.append(t_)
            cm_t = inp.tile([128, NB * 16], bf16, name="cm_t", tag="cm_t")

            warm_s = inp.tile([128, CBW], bf16, name="warm_s", tag="warm_s")
            nc.vector.memset(warm_s[:], 0.0)
            nc.scalar.dma_start(wt_t[0][:], wts[0])
            nc.scalar.dma_start(cm_t[:], cmask[:])
            for t in range(1, NSLOT):
                nc.scalar.dma_start(wt_t[t][:], wts[t])
            # col0/col1 in halves (parallel DMA channels -> earlier first
            # pair); evens on sync, odds on gpsimd, in compute order
            nc.sync.dma_start(col_t[0][:, :4, :], cols[0][:, :4, :])
            nc.sync.dma_start(col_t[0][:, 4:, :], cols[0][:, 4:, :])
            nc.gpsimd.dma_start(col_t[1][:, :4, :], cols[1][:, :4, :])
            nc.gpsimd.dma_start(col_t[1][:, 4:, :], cols[1][:, 4:, :])
            for c in (2, 4, 6, 8, 10, 12):
                nc.sync.dma_start(col_t[c][:], cols[c])
            for c in (3, 5, 7, 9, 11):
                nc.gpsimd.dma_start(col_t[c][:], cols[c])

            racc_t = outp.tile([128, NB], f32, name="racc_t", tag="racc_t")
            cs_psum = pcs.tile([16, CBW], f32, name="cs_psum", tag="cs_psum")

            # PE warm-up during the DMA wait: ramp the clock
            for w in range(6):
                warm_p = pd.tile([128, 2, CBW], f32, name=f"warm{w}", tag="d")
                nc.tensor.matmul(warm_p[:, 0, :], warm_s[:, :128], warm_s[:],
                                 start=True, stop=True)

            nmir = 0
            pend = []          # deferred mirror matmuls: (b, ad2, j)

            def flush_cs(last=False):
                nonlocal nmir
                for b_, ad2_, j_ in pend:
                    nc.tensor.matmul(
                        cs_psum[:],
                        cm_t[:, 16 * b_: 16 * (b_ + 1)],
                        ad2_[:, j_, :],
                        start=(nmir == 0),
                        stop=(last and nmir == NMIR - 1),
                    )
                    nmir += 1
                pend.clear()

            for pi, (blkA, blkB) in enumerate(PAIRS):
                d2 = pd.tile([128, 2, CBW], f32, name=f"d2_{pi}", tag="d")
                # one PSUM accumulation group over both halves of the bank:
                # start zeroes the whole bank (pending-zero), halves then
                # accumulate independently
                for g in range(NG):
                    for j, (t, c) in enumerate((blkA, blkB)):
                        nc.tensor.matmul(
                            d2[:, j, :],
                            wt_t[t][:, 2 * g: 2 * g + 2, :],
                            col_t[c][:, 2 * g: 2 * g + 2, :],
                            start=(g == 0 and j == 0),
                            stop=(g == NG - 1 and j == 1),
                            perf_mode=DR,
                            skip_group_check=True,
                        )
                ad2 = adp.tile([128, 2, CBW], bf16, name=f"ad2_{pi}",
                               tag="ad")
                nc.scalar.activation(ad2[:], d2[:],
                                     mybir.ActivationFunctionType.Abs)
                nc.vector.tensor_reduce(
                    out=racc_t[:, 2 * pi: 2 * pi + 2],
                    in_=ad2[:],
                    axis=mybir.AxisListType.X,
                    op=mybir.AluOpType.add,
                )
                for j, (t, c) in enumerate((blkA, blkB)):
                    if c > CSTART[t]:
                        pend.append((2 * pi + j, ad2, j))
                # batch mirror matmuls of 3 pairs: one weight-switch
                # round-trip on the PE per batch instead of per pair
                flush_cs()
            flush_cs(last=True)

            cs_sb = outp.tile([16, CBW], f32, name="cs_sb", tag="cs_sb")
            nc.scalar.copy(cs_sb[:], cs_psum[:])
            nc.sync.dma_start(cs_out[:], cs_sb[:])
            nc.sync.dma_start(racc_out[:], racc_t[:])

    nc.compile()
    return nc


def _get_compiled():
    global _COMPILED
    if _COMPILED is None:
        _COMPILED = _build()
    return _COMPILED


def _normalize(x):
    n = np.sqrt((x.astype(np.float64) ** 2).sum(-1, keepdims=True))
    return (x / np.maximum(n, EPS)).astype(np.float32)


def _device_rowsums(xnf, xnz):
    """xnf (M, 256), xnz (M, 768) f32 -> S (M,) row sums of |feat-frozen|."""
    global _last_bass_results
    from concourse.bass_utils import run_bass_kernel_spmd

    nc = _get_compiled()

    qf = (xnf * SCALE).astype(F8)                 # (M, 256)
    qz = (xnz * SCALE).astype(F8)                 # (M, 768)
    # K-major chunks, cols zero-padded to MC
    chunks = np.zeros((NK, 128, MC), F8)
    chunks[:2, :, :M] = np.ascontiguousarray(qf.T).reshape(2, 128, M)
    chunks[2:, :, :M] = np.ascontiguousarray(qz.T).reshape(6, 128, M)
    # cols[c, p, k, x] = chunks[k, p, CBW*c + x]
    cols_np = np.ascontiguousarray(
        chunks.reshape(NK, 128, NCB, CBW).transpose(2, 1, 0, 3))
    # weights: frozen chunks sign-flipped; rows use the MR (=3200) padding
    wneg = chunks[:, :, :MR].copy()
    wneg[2:] = (wneg[2:].view(np.uint8) ^ 0x80).view(F8)
    wall = np.ascontiguousarray(
        wneg.reshape(NK, 128, NRT, RT).transpose(2, 1, 0, 3))  # [25,128,8,128]

    in_maps = []
    for cid in range(NCORES):
        wt = np.zeros((NSLOT, 128, NK, RT), F8)
        cm = np.zeros((128, NB, 16), np.float32)
        for t in range(NSLOT):
            r = NCORES * t + cid
            if r < NRT:
                wt[t] = wall[r]
                jd = r // 2
                for b, (bt, c) in enumerate(BLOCKS):
                    if bt == t and c > jd:
                        cm[:, b, c] = 1.0
        in_maps.append({
            "wts": wt,
            "cols": cols_np,
            "cmask": np.ascontiguousarray(
                cm.reshape(128, NB * 16)).astype(BF16),
        })

    res = run_bass_kernel_spmd(nc, in_maps, list(range(NCORES)))
    _last_bass_results = res

    S = np.zeros(MC, np.float64)
    for cid in range(NCORES):
        racc = res.results[cid]["racc"].astype(np.float64)   # (128, 28)
        cs = res.results[cid]["cs"].astype(np.float64)       # (16, 256)
        for b, (t, c) in enumerate(BLOCKS):
            r = NCORES * t + cid
            if r < NRT and c >= r // 2:
                S[RT * r: RT * (r + 1)] += racc[:, b]
        S[:MC] += cs[:NCB].reshape(-1)
    return (S[:M] / (SCALE * SCALE)).astype(np.float32)


def kernel(frozen_embeddings, feature_embeddings, proto_sim, labels):
    fz = np.asarray(frozen_embeddings, dtype=np.float32).reshape(M, D)
    fn = np.asarray(feature_embeddings, dtype=np.float32).reshape(M, NF)
    ps_ = np.asarray(proto_sim, dtype=np.float32)
    lab = np.asarray(labels)

    xnf = _normalize(fn)
    xnz = _normalize(fz)

    # dense part on the 8 NeuronCores
    S = _device_rowsums(xnf, xnz)

    # prototype max/argmax and labels (host, tiny)
    psr = ps_.transpose(0, 2, 1).reshape(M, P)
    mps = psr.max(1)
    pidx = psr.argmax(1)
    ext = np.repeat(lab, N)

    # sparse ranking candidates: only same-argmax-prototype pairs can be nonzero
    cand_vals, cand_flat = [], []
    for p in np.unique(pidx):
        g = np.nonzero(pidx == p)[0]
        s = len(g)
        if s < 2:
            continue
        F = xnf[g] @ xnf[g].T
        Z = xnz[g] @ xnz[g].T
        V = (F - Z) * np.outer(mps[g], mps[g])
        iu, ju = np.triu_indices(s, 1)
        ok = ext[g][iu] != ext[g][ju]
        if ok.any():
            cand_vals.append(V[iu[ok], ju[ok]].astype(np.float64))
            cand_flat.append(g[iu[ok]].astype(np.int64) * M + g[ju[ok]])
    if cand_vals:
        vals = np.concatenate(cand_vals)
        flats = np.concatenate(cand_flat)
    else:
        vals = np.zeros(0)
        flats = np.zeros(0, np.int64)

    # top-5 with lax.top_k tie semantics (desc value, then asc flat index);
    # entries not in the candidate set are exact zeros in the ranking matrix.
    order = np.lexsort((flats, -vals))
    pos = [f for f in order if vals[f] > 0][:K_]
    sel_flats = [int(flats[i]) for i in pos]
    if len(sel_flats) < K_:
        nonzero = set(int(f) for v, f in zip(vals, flats) if v != 0.0)
        f = 0
        while len(sel_flats) < K_:
            if f not in nonzero:
                sel_flats.append(f)
            f += 1
    sel_flats = np.asarray(sel_flats, np.int64)
    rows = sel_flats // M
    cols_sel = sel_flats % M

    out = GAMMA * (S[rows].sum(dtype=np.float64)
                   + S[cols_sel].sum(dtype=np.float64)) / (2 * K_ * M)
    return np.asarray(np.float32(out))
